# revision 13
# baseline (speedup 1.0000x reference)
"""Trainium2 Bass kernel for nn_EqPBC (triplet-feature PBC equalizer).

Data-parallel over 8 NeuronCores: each core handles 8192 samples.
Per core, per chunk of 512 samples (batch on free dim, features on partitions):
  1. DMA [128,82] f32 blocks, cast bf16 (DVE), PE-transpose -> E^T [82,512] bf16
  2. One-hot gather matmuls (PE): En/Em/Emn rows (p,h) split (p, h<128|h>=128)
  3. DVE: S1 = sum_p En_p*conj(Emn_p), S2 = sum_p Em_p*conj(Emn_p),
     X_i = Em_i*S1 + En_i*S2  (complex, bf16)
  4. PE reduction over h with W' = W[i,h]*(0.5 on diag) folded into lhsT
  5. f32 finish: out = E[:,L,:] + Eout * 10^(task0/10)/2  (exact f32 E_L term)

Out-of-bounds Emn indices replicate JAX gather semantics: wrap negatives,
then clamp -> both OOB entries land on index 40.
"""
import numpy as np
import ml_dtypes
from contextlib import ExitStack

# ----- static problem constants (hardcoded; kernel.py must be self-contained) -----
M = 41
L = M // 2
NMODES = 2
B = 65536
NCORES = 8
BC = B // NCORES          # 8192 samples per core
NB = 512                  # samples per chunk
NCHUNK = BC // NB         # 16
THRESH = 1.0 * M // 2
_idx = [(m, n) for m in range(-L, L + 1) for n in range(m, L + 1) if abs(m * n) <= THRESH]
M_ARR = np.array([m for m, n in _idx], dtype=np.int32)
N_ARR = np.array([n for m, n in _idx], dtype=np.int32)
DIAG = np.array([m == n for m, n in _idx])
HDIM = len(_idx)          # 177
HA = 128                  # h-split: a block [0,128), b block [128,177)
HB = HDIM - HA            # 49

bf16 = ml_dtypes.bfloat16


def _gather_cols(idx_arr):
    """Column indices into E^T[82,:] (row f = 2*m + p) for gathered rows (p,h)."""
    src = np.empty((2, HDIM), dtype=np.int64)
    for p in range(2):
        src[p] = 2 * (L + idx_arr) + p
    return src  # [p, h] -> source row in [0,82)


def _build_consts():
    mn = L + M_ARR + N_ARR
    mn = np.clip(np.where(mn < 0, mn + M, mn), 0, M - 1) - L  # jax wrap+clamp
    srcs = {"n": _gather_cols(N_ARR), "m": _gather_cols(M_ARR), "mn": _gather_cols(mn)}
    gmats = {}
    for k, src in srcs.items():
        G = np.zeros((82, 2 * HDIM), dtype=np.float32)
        for p in range(2):
            for h in range(HDIM):
                # output col ordering: [p0a(128) | p0b(49) | p1a(128) | p1b(49)]
                col = p * HDIM + h
                G[src[p, h], col] = 1.0
        gmats[k] = G.astype(bf16)
    return gmats


def _build_wred(Wr, Wi):
    """[177, 8] bf16: cols (i*4+0,1) = (W'r,W'i) for rhs=X_ir;
    cols (i*4+2,3) = (-W'i, W'r) for rhs=X_ii.  W' = W[i]*(0.5 on diag)."""
    scale = np.where(DIAG, 0.5, 1.0).astype(np.float32)
    out = np.zeros((HDIM, 8), dtype=np.float32)
    for i in range(2):
        wr = Wr[i] * scale
        wi = Wi[i] * scale
        out[:, i * 4 + 0] = wr
        out[:, i * 4 + 1] = wi
        out[:, i * 4 + 2] = -wi
        out[:, i * 4 + 3] = wr
    return out.astype(bf16)


def _build_kernel():
    import concourse.bass as bass
    import concourse.bacc as bacc
    import concourse.tile as tile
    import concourse.mybir as mybir

    dt = mybir.dt
    nc = bacc.Bacc("TRN2", target_bir_lowering=False, debug=False, num_devices=NCORES)
    xr = nc.declare_dram_parameter("xr", [BC, 82], dt.float32, isOutput=False)
    xi = nc.declare_dram_parameter("xi", [BC, 82], dt.float32, isOutput=False)
    ti = nc.declare_dram_parameter("ti", [BC, 4], dt.float32, isOutput=False)
    gn_d = nc.declare_dram_parameter("gn", [82, 2 * HDIM], dt.bfloat16, isOutput=False)
    gm_d = nc.declare_dram_parameter("gm", [82, 2 * HDIM], dt.bfloat16, isOutput=False)
    gmn_d = nc.declare_dram_parameter("gmn", [82, 2 * HDIM], dt.bfloat16, isOutput=False)
    wred_d = nc.declare_dram_parameter("wred", [HDIM, 8], dt.bfloat16, isOutput=False)
    id128_d = nc.declare_dram_parameter("id128", [128, 128], dt.float32, isOutput=False)
    id4_d = nc.declare_dram_parameter("id4", [2, 2], dt.float32, isOutput=False)
    out_d = nc.declare_dram_parameter("out", [BC, 4], dt.float32, isOutput=True)

    LN10_10 = float(np.log(10.0) / 10.0)
    LNHALF = float(np.log(0.5))

    with tile.TileContext(nc) as tc, ExitStack() as ctx:
        cpool = ctx.enter_context(tc.tile_pool(name="consts", bufs=1))
        nat_pool = ctx.enter_context(tc.tile_pool(name="nat", bufs=6))
        et_pool = ctx.enter_context(tc.tile_pool(name="et", bufs=2))
        g_pool = ctx.enter_context(tc.tile_pool(name="gath", bufs=2))
        s_pool = ctx.enter_context(tc.tile_pool(name="smid", bufs=2))
        x_pool = ctx.enter_context(tc.tile_pool(name="xmid", bufs=2))
        e_pool = ctx.enter_context(tc.tile_pool(name="eall", bufs=2))
        o_pool = ctx.enter_context(tc.tile_pool(name="outs", bufs=2))
        pt_psum = ctx.enter_context(tc.tile_pool(name="ptp", bufs=2, space="PSUM"))
        pg_psum = ctx.enter_context(tc.tile_pool(name="pgp", bufs=2, space="PSUM"))
        pe_psum = ctx.enter_context(tc.tile_pool(name="pep", bufs=2, space="PSUM"))
        po_psum = ctx.enter_context(tc.tile_pool(name="pop", bufs=1, space="PSUM"))

        # load constants once
        gmats_sb = {}
        for name, d in (("n", gn_d), ("m", gm_d), ("mn", gmn_d)):
            t = cpool.tile([82, 2 * HDIM], dt.bfloat16, tag=f"g{name}")
            nc.gpsimd.dma_start(out=t[:], in_=d[:])
            gmats_sb[name] = t
        wredA = cpool.tile([HA, 8], dt.bfloat16, tag="wredA")
        nc.gpsimd.dma_start(out=wredA[:], in_=wred_d[0:HA, :])
        wredB = cpool.tile([HB, 8], dt.bfloat16, tag="wredB")
        nc.gpsimd.dma_start(out=wredB[:], in_=wred_d[HA:HDIM, :])
        id128 = cpool.tile([128, 128], dt.float32, tag="id128")
        nc.gpsimd.dma_start(out=id128[:], in_=id128_d[:])
        id4 = cpool.tile([2, 2], dt.float32, tag="id4")
        nc.gpsimd.dma_start(out=id4[:], in_=id4_d[:])
        bias_t = cpool.tile([128, 1], dt.float32, tag="biasln")
        nc.vector.memset(bias_t[:], LNHALF)

        # J-slices of gather matrices: [p0a, p0b, p1a, p1b]
        jslices = [(0, HA), (HA, HB), (HDIM, HA), (HDIM + HA, HB)]

        for c in range(NCHUNK):
            b0 = c * NB
            nat = {}
            etT = {}
            for comp, src in (("r", xr), ("i", xi)):
                et = et_pool.tile([82, NB], dt.bfloat16, tag=f"et{comp}")
                etT[comp] = et
                for blk in range(4):
                    t = nat_pool.tile([128, 82], dt.float32, tag=f"nat{comp}")
                    nc.gpsimd.dma_start(out=t[:], in_=src[b0 + blk * 128: b0 + (blk + 1) * 128, :])
                    if blk == 3:
                        nat[comp] = t  # keep last block for E_L columns (see below)
                    nat[(comp, blk)] = t
                    pt = pt_psum.tile([82, 128], dt.float32, tag="tpsum")
                    nc.tensor.transpose(pt[:], t[:], id128[:])
                    nc.scalar.copy(et[:, blk * 128:(blk + 1) * 128], pt[:])

            # gathers: gtile[kind][comp][j] with j in 0..3 = (p0a,p0b,p1a,p1b)
            gt = {}
            for kind in ("n", "m", "mn"):
                for comp in ("r", "i"):
                    for j, (j0, jl) in enumerate(jslices):
                        ps = pg_psum.tile([128, NB], dt.float32, tag="gpsum")
                        nc.tensor.matmul(ps[:jl, :], gmats_sb[kind][:, j0:j0 + jl],
                                         etT[comp][:], start=True, stop=True)
                        sb = g_pool.tile([128, NB], dt.bfloat16, tag=f"g{kind}{comp}{j}")
                        nc.scalar.copy(sb[:jl, :], ps[:jl, :])
                        gt[(kind, comp, j)] = sb

            def TT(op, out, a, b_, rows):
                getattr(nc.vector, op)(out[:rows, :], a[:rows, :], b_[:rows, :])

            # S-stage: S1 = sum_p En_p*conj(Emn_p); S2 = sum_p Em_p*conj(Emn_p)
            # rows j: 0=p0a,1=p0b,2=p1a,3=p1b ; fold p: (0,2)->a, (1,3)->b
            S = {}
            for (sname, kind) in (("S1", "n"), ("S2", "m")):
                for comp in ("r", "i"):
                    prods = []
                    for j in range(4):
                        rows = HA if j % 2 == 0 else HB
                        pa = s_pool.tile([128, NB], dt.bfloat16, tag="ptmpA")
                        pb_ = s_pool.tile([128, NB], dt.bfloat16, tag="ptmpB")
                        if comp == "r":
                            # re: Er*EMNr + Ei*EMNi
                            TT("tensor_mul", pa, gt[(kind, "r", j)], gt[("mn", "r", j)], rows)
                            TT("tensor_mul", pb_, gt[(kind, "i", j)], gt[("mn", "i", j)], rows)
                            q = s_pool.tile([128, NB], dt.bfloat16, tag=f"q{j}")
                            TT("tensor_add", q, pa, pb_, rows)
                        else:
                            # im: Ei*EMNr - Er*EMNi
                            TT("tensor_mul", pa, gt[(kind, "i", j)], gt[("mn", "r", j)], rows)
                            TT("tensor_mul", pb_, gt[(kind, "r", j)], gt[("mn", "i", j)], rows)
                            q = s_pool.tile([128, NB], dt.bfloat16, tag=f"q{j}")
                            TT("tensor_sub", q, pa, pb_, rows)
                        prods.append(q)
                    sa = s_pool.tile([128, NB], dt.bfloat16, tag=f"{sname}{comp}a")
                    TT("tensor_add", sa, prods[0], prods[2], HA)
                    sb2 = s_pool.tile([128, NB], dt.bfloat16, tag=f"{sname}{comp}b")
                    TT("tensor_add", sb2, prods[1], prods[3], HB)
                    S[(sname, comp, 0)] = sa
                    S[(sname, comp, 1)] = sb2

            # X-stage per mode i: X_i = Em_i * S1 + En_i * S2 (complex)
            X = {}
            for i in range(2):
                for comp in ("r", "i"):
                    for hb in range(2):  # 0=a(128), 1=b(49)
                        rows = HA if hb == 0 else HB
                        j = 2 * i + hb  # gather tile index for mode i
                        t1 = x_pool.tile([128, NB], dt.bfloat16, tag="xt1")
                        t2 = x_pool.tile([128, NB], dt.bfloat16, tag="xt2")
                        t3 = x_pool.tile([128, NB], dt.bfloat16, tag="xt3")
                        t4 = x_pool.tile([128, NB], dt.bfloat16, tag="xt4")
                        if comp == "r":
                            TT("tensor_mul", t1, gt[("m", "r", j)], S[("S1", "r", hb)], rows)
                            TT("tensor_mul", t2, gt[("m", "i", j)], S[("S1", "i", hb)], rows)
                            TT("tensor_mul", t3, gt[("n", "r", j)], S[("S2", "r", hb)], rows)
                            TT("tensor_mul", t4, gt[("n", "i", j)], S[("S2", "i", hb)], rows)
                            u = x_pool.tile([128, NB], dt.bfloat16, tag="xu")
                            TT("tensor_sub", u, t1, t2, rows)
                            v = x_pool.tile([128, NB], dt.bfloat16, tag="xv")
                            TT("tensor_sub", v, t3, t4, rows)
                        else:
                            TT("tensor_mul", t1, gt[("m", "r", j)], S[("S1", "i", hb)], rows)
                            TT("tensor_mul", t2, gt[("m", "i", j)], S[("S1", "r", hb)], rows)
                            TT("tensor_mul", t3, gt[("n", "r", j)], S[("S2", "i", hb)], rows)
                            TT("tensor_mul", t4, gt[("n", "i", j)], S[("S2", "r", hb)], rows)
                            u = x_pool.tile([128, NB], dt.bfloat16, tag="xu")
                            TT("tensor_add", u, t1, t2, rows)
                            v = x_pool.tile([128, NB], dt.bfloat16, tag="xv")
                            TT("tensor_add", v, t3, t4, rows)
                        xt = x_pool.tile([128, NB], dt.bfloat16, tag=f"x{i}{comp}{hb}")
                        TT("tensor_add", xt, u, v, rows)
                        X[(i, comp, hb)] = xt

            # reduction: Eout_i = sum_h W'_i[h] * X_i[h] (complex via 4 matmuls)
            eall0 = e_pool.tile([2, NB], dt.float32, tag="eall0")
            eall1 = e_pool.tile([2, NB], dt.float32, tag="eall1")
            eall = [eall0, eall1]
            for i in range(2):
                pe = pe_psum.tile([2, NB], dt.float32, tag="epsum")
                nc.tensor.matmul(pe[:], wredA[:, i * 4:i * 4 + 2], X[(i, "r", 0)][:HA, :],
                                 start=True, stop=False)
                nc.tensor.matmul(pe[:], wredB[:, i * 4:i * 4 + 2], X[(i, "r", 1)][:HB, :],
                                 start=False, stop=False)
                nc.tensor.matmul(pe[:], wredA[:, i * 4 + 2:i * 4 + 4], X[(i, "i", 0)][:HA, :],
                                 start=False, stop=False)
                nc.tensor.matmul(pe[:], wredB[:, i * 4 + 2:i * 4 + 4], X[(i, "i", 1)][:HB, :],
                                 start=False, stop=True)
                nc.scalar.copy(eall[i][:], pe[:])

            # final combine per 128-block
            for blk in range(4):
                po = po_psum.tile([128, 4], dt.float32, tag="opsum")
                nc.tensor.transpose(po[:, 0:2], eall[0][:, blk * 128:(blk + 1) * 128], id4[:])
                nc.tensor.transpose(po[:, 2:4], eall[1][:, blk * 128:(blk + 1) * 128], id4[:])
                tit = o_pool.tile([128, 4], dt.float32, tag="tit")
                nc.gpsimd.dma_start(out=tit[:], in_=ti[b0 + blk * 128: b0 + (blk + 1) * 128, :])
                pcol = o_pool.tile([128, 1], dt.float32, tag="pcol")
                import concourse.mybir as _mb
                nc.scalar.activation(pcol[:], tit[:, 0:1], _mb.ActivationFunctionType.Exp,
                                     bias=bias_t[:], scale=LN10_10)
                ot = o_pool.tile([128, 4], dt.float32, tag="ot")
                nc.vector.tensor_scalar_mul(ot[:], po[:], pcol[:])
                # add exact E_L columns: out cols (0,2) += xr_nat[:, 40:42]; (1,3) += xi_nat
                nc.vector.tensor_add(ot[:, 0:4:2], ot[:, 0:4:2], nat[("r", blk)][:, 2 * L:2 * L + 2])
                nc.vector.tensor_add(ot[:, 1:4:2], ot[:, 1:4:2], nat[("i", blk)][:, 2 * L:2 * L + 2])
                nc.sync.dma_start(out=out_d[b0 + blk * 128: b0 + (blk + 1) * 128, :], in_=ot[:])

    nc.compile()
    return nc


_CACHE = {}


def kernel(xr, xi, task_info, Wr, Wi):
    from concourse.bass_utils import run_bass_kernel_spmd

    xr = np.ascontiguousarray(np.asarray(xr, dtype=np.float32)).reshape(B, 82)
    xi = np.ascontiguousarray(np.asarray(xi, dtype=np.float32)).reshape(B, 82)
    task_info = np.ascontiguousarray(np.asarray(task_info, dtype=np.float32))
    gm = _build_consts()
    wred = _build_wred(np.asarray(Wr, dtype=np.float32), np.asarray(Wi, dtype=np.float32))
    id128 = np.eye(128, dtype=np.float32)
    id4 = np.eye(2, dtype=np.float32)

    if "nc" not in _CACHE:
        _CACHE["nc"] = _build_kernel()
    nc = _CACHE["nc"]

    in_maps = []
    for core in range(NCORES):
        s = slice(core * BC, (core + 1) * BC)
        in_maps.append({
            "xr": xr[s], "xi": xi[s], "ti": task_info[s],
            "gn": gm["n"], "gm": gm["m"], "gmn": gm["mn"],
            "wred": wred, "id128": id128, "id4": id4,
        })
    res = run_bass_kernel_spmd(nc, in_maps, list(range(NCORES)))
    outs = [res.results[i]["out"] for i in range(NCORES)]
    full = np.concatenate(outs, axis=0)  # [B, 4]
    return full.reshape(B, NMODES, 2).astype(np.float32)


# revision 18
# speedup vs baseline: 1.1646x; 1.1646x over previous
"""Trainium2 Bass kernel for nn_EqPBC (triplet-feature PBC equalizer).

Data-parallel over 8 NeuronCores: each core handles 8192 samples.
Per core, per chunk of 512 samples (batch on free dim, features on partitions):
  1. DMA [128,82] f32 blocks, cast bf16 (DVE), PE-transpose -> E^T [82,512] bf16
  2. One-hot gather matmuls (PE): En/Em/Emn rows (p,h) split (p, h<128|h>=128)
  3. DVE: S1 = sum_p En_p*conj(Emn_p), S2 = sum_p Em_p*conj(Emn_p),
     X_i = Em_i*S1 + En_i*S2  (complex, bf16)
  4. PE reduction over h with W' = W[i,h]*(0.5 on diag) folded into lhsT
  5. f32 finish: out = E[:,L,:] + Eout * 10^(task0/10)/2  (exact f32 E_L term)

Out-of-bounds Emn indices replicate JAX gather semantics: wrap negatives,
then clamp -> both OOB entries land on index 40.
"""
import numpy as np
import ml_dtypes
from contextlib import ExitStack

# ----- static problem constants (hardcoded; kernel.py must be self-contained) -----
M = 41
L = M // 2
NMODES = 2
B = 65536
NCORES = 8
BC = B // NCORES          # 8192 samples per core
NB = 512                  # samples per chunk
NCHUNK = BC // NB         # 16
THRESH = 1.0 * M // 2
_idx = [(m, n) for m in range(-L, L + 1) for n in range(m, L + 1) if abs(m * n) <= THRESH]
M_ARR = np.array([m for m, n in _idx], dtype=np.int32)
N_ARR = np.array([n for m, n in _idx], dtype=np.int32)
DIAG = np.array([m == n for m, n in _idx])
HDIM = len(_idx)          # 177
HA = 128                  # h-split: a block [0,128), b block [128,177)
HB = HDIM - HA            # 49

bf16 = ml_dtypes.bfloat16


def _gather_cols(idx_arr):
    """Column indices into E^T[82,:] (row f = 2*m + p) for gathered rows (p,h)."""
    src = np.empty((2, HDIM), dtype=np.int64)
    for p in range(2):
        src[p] = 2 * (L + idx_arr) + p
    return src  # [p, h] -> source row in [0,82)


def _build_consts():
    mn = L + M_ARR + N_ARR
    mn = np.clip(np.where(mn < 0, mn + M, mn), 0, M - 1) - L  # jax wrap+clamp
    srcs = {"n": _gather_cols(N_ARR), "m": _gather_cols(M_ARR), "mn": _gather_cols(mn)}
    gmats = {}
    for k, src in srcs.items():
        G = np.zeros((82, 2 * HDIM), dtype=np.float32)
        for p in range(2):
            for h in range(HDIM):
                # output col ordering: [p0a(128) | p0b(49) | p1a(128) | p1b(49)]
                col = p * HDIM + h
                G[src[p, h], col] = 1.0
        gmats[k] = G.astype(bf16)
    return gmats


def _build_wred(Wr, Wi):
    """[177, 8] bf16: cols (i*4+0,1) = (W'r,W'i) for rhs=X_ir;
    cols (i*4+2,3) = (-W'i, W'r) for rhs=X_ii.  W' = W[i]*(0.5 on diag)."""
    scale = np.where(DIAG, 0.5, 1.0).astype(np.float32)
    out = np.zeros((HDIM, 8), dtype=np.float32)
    for i in range(2):
        wr = Wr[i] * scale
        wi = Wi[i] * scale
        out[:, i * 4 + 0] = wr
        out[:, i * 4 + 1] = wi
        out[:, i * 4 + 2] = -wi
        out[:, i * 4 + 3] = wr
    return out.astype(bf16)


def _build_kernel():
    import concourse.bass as bass
    import concourse.bacc as bacc
    import concourse.tile as tile
    import concourse.mybir as mybir

    dt = mybir.dt
    nc = bacc.Bacc("TRN2", target_bir_lowering=False, debug=False, num_devices=NCORES)
    xr = nc.declare_dram_parameter("xr", [BC, 82], dt.float32, isOutput=False)
    xi = nc.declare_dram_parameter("xi", [BC, 82], dt.float32, isOutput=False)
    ti = nc.declare_dram_parameter("ti", [BC, 4], dt.float32, isOutput=False)
    gn_d = nc.declare_dram_parameter("gn", [82, 2 * HDIM], dt.bfloat16, isOutput=False)
    gm_d = nc.declare_dram_parameter("gm", [82, 2 * HDIM], dt.bfloat16, isOutput=False)
    gmn_d = nc.declare_dram_parameter("gmn", [82, 2 * HDIM], dt.bfloat16, isOutput=False)
    wred_d = nc.declare_dram_parameter("wred", [HDIM, 8], dt.bfloat16, isOutput=False)
    id128_d = nc.declare_dram_parameter("id128", [128, 128], dt.float32, isOutput=False)
    id4_d = nc.declare_dram_parameter("id4", [2, 2], dt.float32, isOutput=False)
    out_d = nc.declare_dram_parameter("out", [BC, 4], dt.float32, isOutput=True)

    LN10_10 = float(np.log(10.0) / 10.0)
    LNHALF = float(np.log(0.5))

    with tile.TileContext(nc) as tc, ExitStack() as ctx:
        cpool = ctx.enter_context(tc.tile_pool(name="consts", bufs=1))
        nat_pool = ctx.enter_context(tc.tile_pool(name="nat", bufs=6))
        et_pool = ctx.enter_context(tc.tile_pool(name="et", bufs=2))
        g_pool = ctx.enter_context(tc.tile_pool(name="gath", bufs=2))
        s_pool = ctx.enter_context(tc.tile_pool(name="smid", bufs=3))
        tmp_pool = ctx.enter_context(tc.tile_pool(name="tmps", bufs=4))
        x_pool = ctx.enter_context(tc.tile_pool(name="xmid", bufs=3))
        e_pool = ctx.enter_context(tc.tile_pool(name="eall", bufs=2))
        o_pool = ctx.enter_context(tc.tile_pool(name="outs", bufs=2))
        pt_psum = ctx.enter_context(tc.tile_pool(name="ptp", bufs=2, space="PSUM"))
        pg_psum = ctx.enter_context(tc.tile_pool(name="pgp", bufs=3, space="PSUM"))
        pe_psum = ctx.enter_context(tc.tile_pool(name="pep", bufs=2, space="PSUM"))
        po_psum = ctx.enter_context(tc.tile_pool(name="pop", bufs=1, space="PSUM"))

        # load constants once
        gmats_sb = {}
        for name, d in (("n", gn_d), ("m", gm_d), ("mn", gmn_d)):
            t = cpool.tile([82, 2 * HDIM], dt.bfloat16, tag=f"g{name}")
            nc.gpsimd.dma_start(out=t[:], in_=d[:])
            gmats_sb[name] = t
        wredA = cpool.tile([HA, 8], dt.bfloat16, tag="wredA")
        nc.gpsimd.dma_start(out=wredA[:], in_=wred_d[0:HA, :])
        wredB = cpool.tile([HB, 8], dt.bfloat16, tag="wredB")
        nc.gpsimd.dma_start(out=wredB[:], in_=wred_d[HA:HDIM, :])
        id128 = cpool.tile([128, 128], dt.float32, tag="id128")
        nc.gpsimd.dma_start(out=id128[:], in_=id128_d[:])
        id4 = cpool.tile([2, 2], dt.float32, tag="id4")
        nc.gpsimd.dma_start(out=id4[:], in_=id4_d[:])
        bias_t = cpool.tile([128, 1], dt.float32, tag="biasln")
        nc.vector.memset(bias_t[:], LNHALF)

        # J-slices of gather matrices: [p0a, p0b, p1a, p1b]
        jslices = [(0, HA), (HA, HB), (HDIM, HA), (HDIM + HA, HB)]

        for c in range(NCHUNK):
            b0 = c * NB
            nat = {}
            etT = {}
            for comp, src in (("r", xr), ("i", xi)):
                et = et_pool.tile([82, NB], dt.bfloat16, tag=f"et{comp}")
                etT[comp] = et
                for blk in range(4):
                    t = nat_pool.tile([128, 82], dt.float32, tag=f"nat{comp}")
                    nc.gpsimd.dma_start(out=t[:], in_=src[b0 + blk * 128: b0 + (blk + 1) * 128, :])
                    if blk == 3:
                        nat[comp] = t  # keep last block for E_L columns (see below)
                    nat[(comp, blk)] = t
                    pt = pt_psum.tile([82, 128], dt.float32, tag="tpsum")
                    nc.tensor.transpose(pt[:], t[:], id128[:])
                    nc.scalar.copy(et[:, blk * 128:(blk + 1) * 128], pt[:])

            # gathers: gtile[kind][comp][j] with j in 0..3 = (p0a,p0b,p1a,p1b)
            gt = {}
            for kind in ("n", "m", "mn"):
                for comp in ("r", "i"):
                    for j, (j0, jl) in enumerate(jslices):
                        ps = pg_psum.tile([128, NB], dt.float32, tag="gpsum")
                        nc.tensor.matmul(ps[:jl, :], gmats_sb[kind][:, j0:j0 + jl],
                                         etT[comp][:], start=True, stop=True)
                        sb = g_pool.tile([128, NB], dt.bfloat16, tag=f"g{kind}{comp}{j}")
                        nc.scalar.copy(sb[:jl, :], ps[:jl, :])
                        gt[(kind, comp, j)] = sb

            def TT(op, out, a, b_, rows, eng=None):
                getattr(eng or nc.vector, op)(out[:rows, :], a[:rows, :], b_[:rows, :])

            # S-stage: S1 = sum_p En_p*conj(Emn_p); S2 = sum_p Em_p*conj(Emn_p)
            # rows j: 0=p0a,1=p0b,2=p1a,3=p1b ; fold p: (0,2)->a, (1,3)->b
            S = {}
            for (sname, kind) in (("S1", "n"), ("S2", "m")):
                for comp in ("r", "i"):
                    prods = []
                    for j in range(4):
                        rows = HA if j % 2 == 0 else HB
                        pa = tmp_pool.tile([128, NB], dt.bfloat16, tag="ptmpA")
                        pb_ = tmp_pool.tile([128, NB], dt.bfloat16, tag="ptmpB")
                        if comp == "r":
                            # re: Er*EMNr + Ei*EMNi
                            TT("tensor_mul", pa, gt[(kind, "r", j)], gt[("mn", "r", j)], rows)
                            TT("tensor_mul", pb_, gt[(kind, "i", j)], gt[("mn", "i", j)], rows)
                            q = tmp_pool.tile([128, NB], dt.bfloat16, tag=f"q{j}")
                            TT("tensor_add", q, pa, pb_, rows)
                        else:
                            # im: Ei*EMNr - Er*EMNi
                            TT("tensor_mul", pa, gt[(kind, "i", j)], gt[("mn", "r", j)], rows)
                            TT("tensor_mul", pb_, gt[(kind, "r", j)], gt[("mn", "i", j)], rows)
                            q = tmp_pool.tile([128, NB], dt.bfloat16, tag=f"q{j}")
                            TT("tensor_sub", q, pa, pb_, rows)
                        prods.append(q)
                    sa = s_pool.tile([128, NB], dt.bfloat16, tag=f"{sname}{comp}a")
                    TT("tensor_add", sa, prods[0], prods[2], HA)
                    sb2 = s_pool.tile([128, NB], dt.bfloat16, tag=f"{sname}{comp}b")
                    TT("tensor_add", sb2, prods[1], prods[3], HB)
                    S[(sname, comp, 0)] = sa
                    S[(sname, comp, 1)] = sb2

            # X-stage per mode i: X_i = Em_i * S1 + En_i * S2 (complex)
            X = {}
            for i in range(2):
                xeng = nc.vector
                for comp in ("r", "i"):
                    for hb in range(2):  # 0=a(128), 1=b(49)
                        rows = HA if hb == 0 else HB
                        j = 2 * i + hb  # gather tile index for mode i
                        t1 = tmp_pool.tile([128, NB], dt.bfloat16, tag="xt1")
                        t2 = tmp_pool.tile([128, NB], dt.bfloat16, tag="xt2")
                        t3 = tmp_pool.tile([128, NB], dt.bfloat16, tag="xt3")
                        t4 = tmp_pool.tile([128, NB], dt.bfloat16, tag="xt4")
                        if comp == "r":
                            TT("tensor_mul", t1, gt[("m", "r", j)], S[("S1", "r", hb)], rows, eng=xeng)
                            TT("tensor_mul", t2, gt[("m", "i", j)], S[("S1", "i", hb)], rows, eng=xeng)
                            TT("tensor_mul", t3, gt[("n", "r", j)], S[("S2", "r", hb)], rows, eng=xeng)
                            TT("tensor_mul", t4, gt[("n", "i", j)], S[("S2", "i", hb)], rows, eng=xeng)
                            u = tmp_pool.tile([128, NB], dt.bfloat16, tag="xu")
                            TT("tensor_sub", u, t1, t2, rows, eng=xeng)
                            v = tmp_pool.tile([128, NB], dt.bfloat16, tag="xv")
                            TT("tensor_sub", v, t3, t4, rows, eng=xeng)
                        else:
                            TT("tensor_mul", t1, gt[("m", "r", j)], S[("S1", "i", hb)], rows, eng=xeng)
                            TT("tensor_mul", t2, gt[("m", "i", j)], S[("S1", "r", hb)], rows, eng=xeng)
                            TT("tensor_mul", t3, gt[("n", "r", j)], S[("S2", "i", hb)], rows, eng=xeng)
                            TT("tensor_mul", t4, gt[("n", "i", j)], S[("S2", "r", hb)], rows, eng=xeng)
                            u = tmp_pool.tile([128, NB], dt.bfloat16, tag="xu")
                            TT("tensor_add", u, t1, t2, rows, eng=xeng)
                            v = tmp_pool.tile([128, NB], dt.bfloat16, tag="xv")
                            TT("tensor_add", v, t3, t4, rows, eng=xeng)
                        xt = x_pool.tile([128, NB], dt.bfloat16, tag=f"x{i}{comp}{hb}")
                        TT("tensor_add", xt, u, v, rows, eng=xeng)
                        X[(i, comp, hb)] = xt

            # reduction: Eout_i = sum_h W'_i[h] * X_i[h] (complex via 4 matmuls)
            eall0 = e_pool.tile([2, NB], dt.float32, tag="eall0")
            eall1 = e_pool.tile([2, NB], dt.float32, tag="eall1")
            eall = [eall0, eall1]
            for i in range(2):
                pe = pe_psum.tile([2, NB], dt.float32, tag="epsum")
                nc.tensor.matmul(pe[:], wredA[:, i * 4:i * 4 + 2], X[(i, "r", 0)][:HA, :],
                                 start=True, stop=False)
                nc.tensor.matmul(pe[:], wredB[:, i * 4:i * 4 + 2], X[(i, "r", 1)][:HB, :],
                                 start=False, stop=False)
                nc.tensor.matmul(pe[:], wredA[:, i * 4 + 2:i * 4 + 4], X[(i, "i", 0)][:HA, :],
                                 start=False, stop=False)
                nc.tensor.matmul(pe[:], wredB[:, i * 4 + 2:i * 4 + 4], X[(i, "i", 1)][:HB, :],
                                 start=False, stop=True)
                nc.scalar.copy(eall[i][:], pe[:])

            # final combine per 128-block
            for blk in range(4):
                po = po_psum.tile([128, 4], dt.float32, tag="opsum")
                nc.tensor.transpose(po[:, 0:2], eall[0][:, blk * 128:(blk + 1) * 128], id4[:])
                nc.tensor.transpose(po[:, 2:4], eall[1][:, blk * 128:(blk + 1) * 128], id4[:])
                tit = o_pool.tile([128, 4], dt.float32, tag="tit")
                nc.gpsimd.dma_start(out=tit[:], in_=ti[b0 + blk * 128: b0 + (blk + 1) * 128, :])
                pcol = o_pool.tile([128, 1], dt.float32, tag="pcol")
                import concourse.mybir as _mb
                nc.scalar.activation(pcol[:], tit[:, 0:1], _mb.ActivationFunctionType.Exp,
                                     bias=bias_t[:], scale=LN10_10)
                ot = o_pool.tile([128, 4], dt.float32, tag="ot")
                nc.vector.tensor_scalar_mul(ot[:], po[:], pcol[:])
                # add exact E_L columns: out cols (0,2) += xr_nat[:, 40:42]; (1,3) += xi_nat
                nc.vector.tensor_add(ot[:, 0:4:2], ot[:, 0:4:2], nat[("r", blk)][:, 2 * L:2 * L + 2])
                nc.vector.tensor_add(ot[:, 1:4:2], ot[:, 1:4:2], nat[("i", blk)][:, 2 * L:2 * L + 2])
                nc.sync.dma_start(out=out_d[b0 + blk * 128: b0 + (blk + 1) * 128, :], in_=ot[:])

    nc.compile()
    return nc


_CACHE = {}


def kernel(xr, xi, task_info, Wr, Wi):
    from concourse.bass_utils import run_bass_kernel_spmd

    xr = np.ascontiguousarray(np.asarray(xr, dtype=np.float32)).reshape(B, 82)
    xi = np.ascontiguousarray(np.asarray(xi, dtype=np.float32)).reshape(B, 82)
    task_info = np.ascontiguousarray(np.asarray(task_info, dtype=np.float32))
    gm = _build_consts()
    wred = _build_wred(np.asarray(Wr, dtype=np.float32), np.asarray(Wi, dtype=np.float32))
    id128 = np.eye(128, dtype=np.float32)
    id4 = np.eye(2, dtype=np.float32)

    if "nc" not in _CACHE:
        _CACHE["nc"] = _build_kernel()
    nc = _CACHE["nc"]

    in_maps = []
    for core in range(NCORES):
        s = slice(core * BC, (core + 1) * BC)
        in_maps.append({
            "xr": xr[s], "xi": xi[s], "ti": task_info[s],
            "gn": gm["n"], "gm": gm["m"], "gmn": gm["mn"],
            "wred": wred, "id128": id128, "id4": id4,
        })
    res = run_bass_kernel_spmd(nc, in_maps, list(range(NCORES)))
    outs = [res.results[i]["out"] for i in range(NCORES)]
    full = np.concatenate(outs, axis=0)  # [B, 4]
    return full.reshape(B, NMODES, 2).astype(np.float32)
